# revision 9
# baseline (speedup 1.0000x reference)
"""Trainium2 Bass kernel for nn_GroupedLinear (16-group LayerNorm+Linear).

Problem: x [1024, 8, 64, 64] fp32; per group g (16 groups of 64 channels):
  X_g = contiguous 2M-element chunk g viewed row-major as [32768, 64]
  Y_g = LayerNorm(X_g) * gamma_g + beta_g  @ W_g^T + b_g      [32768, 64]
  out chunk g = Y_g^T  (contiguous [64, 32768] block of the output)

Sharding: expert-parallel, 2 groups per core across 8 cores; no collectives.

v3: bf16 end-to-end on the wire (host casts x to bf16, device stores bf16
output, host upcasts). Halves both DMA directions vs fp32. Engine balance:
DVE does bn_stats (16 1-seg ops/macro, HW limit) + the rstd multiply in 2x
mode (all-bf16 unit-stride pair APs); GpSimd does the mean subtract (its
only bulk job, fp32-mu broadcast); ACT does rstd prep + PSUM->SBUF copies.
Per-core dataflow (2048-row macro-tile, 16 iterations):
  DMA in bf16 [128p, 2g, 16blk, 64ch] (2KB contiguous per (p,g))
  -> 16x bn_stats ([p,64c,2g] interleaved: even=g0, odd=g1 -> [p,6])
  -> ACT rstd2 = 1/sqrt(M2/64+eps) bf16 pair-duplicated [p,g,b,2]
  -> GpSimd sub -> DVE 2x mul -> xn [p, g, blk, c]
  -> per 1024-col half: 8 PE transposes -> PSUM bf16; ACT copy -> SBUF
     -> 2 matmuls (N=512, block-diag gamma-folded W) -> PSUM f32
     -> bias-add + row un-permute PSUM->SBUF bf16 (ACT / DVE alternating)
  -> DMA out bf16 (4KB contiguous per partition)
Tail: one fp32 is_transpose bit-pattern probe (tpat) to test whether PE
transpose preserves raw bits (incl. denormals) -> gates a future packing
optimization. Read back via uint32-bitcast DVE copy so nothing flushes.
"""

import sys

for _p in ("/opt/trn_rl_repo", "/opt/pypackages"):
    if _p not in sys.path:
        sys.path.insert(0, _p)

import numpy as np
import ml_dtypes

G_TOTAL = 16
N_CORES = 8
G_PER_CORE = G_TOTAL // N_CORES  # 2
IN_G = 64
OUT_G = 64
ROWS = 8 * 64 * 64  # 32768 rows per group
MACRO = 2048  # rows per macro-tile
NB = MACRO // 128  # 16 row-blocks per macro (row = p*NB + b)
NMAC = ROWS // MACRO  # 16
EPS = 1e-6

_CACHE = {}
_LAST_RESULTS = None


def _build_bass(rep=1):
    import concourse.bacc as bacc
    import concourse.bass as bass
    import concourse.tile as tile
    from concourse import mybir

    nc = bacc.Bacc(None, target_bir_lowering=False)

    x = nc.dram_tensor("x", [G_PER_CORE, ROWS, IN_G], mybir.dt.bfloat16,
                       kind="ExternalInput")
    wb = nc.dram_tensor("wb", [128, 128], mybir.dt.bfloat16,
                        kind="ExternalInput")
    tb = nc.dram_tensor("tb", [128, 1], mybir.dt.float32,
                        kind="ExternalInput")
    ident = nc.dram_tensor("ident", [128, 128], mybir.dt.bfloat16,
                           kind="ExternalInput")
    identf = nc.dram_tensor("identf", [128, 128], mybir.dt.float32,
                            kind="ExternalInput")
    tpat_in = nc.dram_tensor("tpat_in", [128, 128], mybir.dt.float32,
                             kind="ExternalInput")
    out = nc.dram_tensor("out", [128, ROWS], mybir.dt.bfloat16,
                         kind="ExternalOutput")
    tpat_out = nc.dram_tensor("tpat_out", [128, 128], mybir.dt.float32,
                              kind="ExternalOutput")

    F = mybir.ActivationFunctionType

    with tile.TileContext(nc) as tc:
        with (
            tc.tile_pool(name="singles", bufs=1) as singles,
            tc.tile_pool(name="xload", bufs=3) as xload,
            tc.tile_pool(name="statp", bufs=3) as statp,
            tc.tile_pool(name="rstdp", bufs=3) as rstdp,
            tc.tile_pool(name="xnp", bufs=2) as xnp,
            tc.tile_pool(name="xtsp", bufs=3) as xtsp,
            tc.tile_pool(name="youtp", bufs=2) as youtp,
            tc.tile_pool(name="xtpp", bufs=2, space="PSUM") as xtpp,
            tc.tile_pool(name="ypp", bufs=4, space="PSUM") as ypp,
        ):
            sb_wb = singles.tile([128, 128], mybir.dt.bfloat16)
            sb_tb = singles.tile([128, 1], mybir.dt.float32)
            sb_id = singles.tile([128, 128], mybir.dt.bfloat16)
            sb_eps = singles.tile([128, 1], mybir.dt.float32)
            nc.sync.dma_start(out=sb_wb, in_=wb[:, :])
            nc.sync.dma_start(out=sb_tb, in_=tb[:, :])
            nc.sync.dma_start(out=sb_id, in_=ident[:, :])
            nc.vector.memset(sb_eps, EPS)

            for m in range(NMAC * rep):
                m = m % NMAC
                r0 = m * MACRO
                # ---- load (one DMA, both groups): partition p holds rows
                # NB*p .. NB*p+NB-1 of each group (2KB contiguous per (p,g))
                x_t = xload.tile([128, G_PER_CORE, NB, IN_G],
                                 mybir.dt.bfloat16)
                nc.sync.dma_start(
                    out=x_t,
                    in_=x[:, r0:r0 + MACRO, :].rearrange(
                        "g (p b) c -> p g b c", p=128),
                )

                # ---- stats: 16 single-segment bn_stats (HW limit: 6 out
                # els/partition). stream [c][g] with g innermost: even
                # positions = g0, odd = g1 -> out [p, 6] per block =
                # [cnt0, mu0, M2_0, cnt1, mu1, M2_1]
                st = statp.tile([128, NB, 6], mybir.dt.float32)
                for bb in range(NB):
                    in3 = x_t[:, :, bb, :].rearrange("p g c -> p c g")
                    nc.vector.add_instruction(
                        mybir.InstBNStats(
                            name=nc.get_next_instruction_name(),
                            ins=[nc.vector.lower_ap(in3)],
                            outs=[nc.vector.lower_ap(st[:, bb, :])],
                        )
                    )
                # rstd2 = 1/sqrt(M2/64 + eps), bf16, pair-duplicated so the
                # DVE multiply sees a unit-stride innermost dim (2x mode).
                # layout [128, b, g, 2] (b-major, matching xn iteration)
                rstd2 = rstdp.tile([128, NB, G_PER_CORE, 2],
                                   mybir.dt.bfloat16)
                st_ap = st[:, :, :]
                m2_in = bass.AP(
                    tensor=st_ap.tensor, offset=st_ap.offset + 2,
                    ap=[st_ap.ap[0], [6, NB], [3, G_PER_CORE], [0, 2]],
                )
                nc.scalar.activation(out=rstd2, in_=m2_in,
                                     func=F.Abs_reciprocal_sqrt,
                                     bias=sb_eps[:, 0:1],
                                     scale=1.0 / IN_G)

                # ---- normalize: sub on GpSimd (fp32 mu broadcast), then
                # mul on DVE in 2x mode (all-bf16, unit-stride pairs).
                # xn layout [p, b, g, c]: stripe b is one contiguous 128-el
                # free range (needed: matmul stationary wants 1 free dim).
                xn = xnp.tile([128, NB, G_PER_CORE, IN_G], mybir.dt.bfloat16)
                xn_v = xn.rearrange("p b g c -> p g b c")
                mu_b = bass.AP(
                    tensor=st_ap.tensor, offset=st_ap.offset + 1,
                    ap=[st_ap.ap[0], [3, G_PER_CORE], [6, NB], [0, IN_G]],
                )
                nc.gpsimd.tensor_sub(xn_v, x_t, mu_b)
                # DVE 2x multiply: all APs [p][(b g) 32][c32][pair 2]
                xn_ap = xn[:, :, :, :]
                xn_p = bass.AP(
                    tensor=xn_ap.tensor, offset=xn_ap.offset,
                    ap=[xn_ap.ap[0], [IN_G, G_PER_CORE * NB],
                        [2, IN_G // 2], [1, 2]],
                )
                rstd2_ap = rstd2[:, :, :, :]
                rstd_p = bass.AP(
                    tensor=rstd2_ap.tensor, offset=rstd2_ap.offset,
                    ap=[rstd2_ap.ap[0], [2, G_PER_CORE * NB],
                        [0, IN_G // 2], [1, 2]],
                )
                nc.vector.tensor_mul(xn_p, xn_p, rstd_p)

                # ---- per half-macro: transpose 8 stripes, copy, matmul,
                # bias+unpermute copy out
                y_t = youtp.tile([128, MACRO], mybir.dt.bfloat16)
                yt_qb = y_t.rearrange("p (q b) -> p b q", b=NB)
                for h in range(2):
                    xtp = xtpp.tile([128, 1024], mybir.dt.bfloat16)
                    for s in range(8):
                        nc.tensor.transpose(
                            out=xtp[:, s * 128:(s + 1) * 128],
                            in_=xn[:, 8 * h + s, :, :].rearrange(
                                "p g c -> p (g c)"),
                            identity=sb_id,
                        )
                    xts = xtsp.tile([128, 1024], mybir.dt.bfloat16)
                    nc.scalar.activation(out=xts, in_=xtp, func=F.Copy)
                    for j in range(2):
                        yp = ypp.tile([128, 512], mybir.dt.float32)
                        nc.tensor.matmul(yp, lhsT=sb_wb,
                                         rhs=xts[:, j * 512:(j + 1) * 512],
                                         start=True, stop=True)
                        # psum free (s', q) -> row q*NB + (8h + 4j + s')
                        yp_v = yp.rearrange("p (s q) -> p s q", q=128)
                        yt_v = yt_qb[:, 8 * h + 4 * j:8 * h + 4 * j + 4, :]
                        nc.scalar.activation(out=yt_v, in_=yp_v,
                                             func=F.Identity,
                                             bias=sb_tb[:, 0:1],
                                             scale=1.0)

                nc.sync.dma_start(out=out[:, r0:r0 + MACRO], in_=y_t)

            # ---- bit-pattern probe: fp32 is_transpose, uint32 readout
            sb_tp = singles.tile([128, 128], mybir.dt.float32)
            sb_idf = singles.tile([128, 128], mybir.dt.float32)
            sb_tpo = singles.tile([128, 128], mybir.dt.float32)
            nc.sync.dma_start(out=sb_tp, in_=tpat_in[:, :])
            nc.sync.dma_start(out=sb_idf, in_=identf[:, :])
            with tc.tile_pool(name="tpp", bufs=1, space="PSUM") as tpp:
                tp_ps = tpp.tile([128, 128], mybir.dt.float32)
                nc.tensor.transpose(out=tp_ps, in_=sb_tp, identity=sb_idf)
                nc.vector.tensor_copy(
                    sb_tpo.bitcast(mybir.dt.uint32),
                    tp_ps.bitcast(mybir.dt.uint32))
            nc.sync.dma_start(out=tpat_out[:, :], in_=sb_tpo)

    nc.finalize()
    return nc


def _get_nc(rep=1):
    key = ("nc", rep)
    if key not in _CACHE:
        _CACHE[key] = _build_bass(rep)
    return _CACHE[key]


def _tpat_patterns():
    """Bit patterns for the transpose probe: packed-bf16-pair shapes,
    denormals (high half zero), plus deterministic pseudo-random bits."""
    rng = np.random.RandomState(12345)
    u = rng.randint(0, 1 << 16, size=(128, 128), dtype=np.uint32)
    u = (u << 16) | rng.randint(0, 1 << 16, size=(128, 128), dtype=np.uint32)
    # force interesting cases: row 0 denormal-high (top 16 bits zero),
    # row 1 zero-low, row 2 classic packed pair (0x3F80 | 0xBF80)
    u[0, :] = u[0, :] & 0x0000FFFF
    u[1, :] = u[1, :] & 0xFFFF0000
    u[2, :] = 0x3F80BF80
    # avoid NaN/Inf high halves elsewhere (clamp exponent field below 0xFF)
    exp = (u >> 23) & 0xFF
    u = np.where(exp == 0xFF, u & ~np.uint32(0x00800000), u)
    return u


def _make_in_maps(x, ln_gamma, ln_beta, W, b):
    bf16 = ml_dtypes.bfloat16
    xg = x.reshape(G_TOTAL, ROWS, IN_G)
    tpat = _tpat_patterns().view(np.float32)
    ident_bf = np.eye(128, dtype=np.float32).astype(bf16)
    ident_f = np.eye(128, dtype=np.float32)
    in_maps = []
    for c in range(N_CORES):
        gs = [G_PER_CORE * c + g for g in range(G_PER_CORE)]
        wbc = np.zeros((128, 128), np.float32)
        tbc = np.zeros((128, 1), np.float32)
        for g_local, g in enumerate(gs):
            Wp = W[g] * ln_gamma[g][None, :]  # [out, in] gamma folded
            lo = g_local * 64
            wbc[lo:lo + 64, lo:lo + 64] = Wp.T  # lhsT[k=in, m=out]
            tbc[lo:lo + 64, 0] = W[g] @ ln_beta[g] + b[g]
        in_maps.append({
            "x": np.ascontiguousarray(xg[gs[0]:gs[-1] + 1]).astype(bf16),
            "wb": wbc.astype(bf16),
            "tb": tbc,
            "ident": ident_bf,
            "identf": ident_f,
            "tpat_in": tpat,
        })
    return in_maps


def _run(in_maps, trace=False):
    from concourse.bass_utils import run_bass_kernel_spmd
    global _LAST_RESULTS
    nc = _get_nc()
    res = run_bass_kernel_spmd(nc, in_maps, list(range(N_CORES)),
                               trace=trace)
    _LAST_RESULTS = res
    return res


def bench(in_maps, rep, iters=12):
    """Time repeated on-device executions of the rep-times-unrolled kernel."""
    import time
    import jax
    import jax.numpy as jnp
    import numpy as np_
    from jax.sharding import Mesh, PartitionSpec
    from jax.experimental.shard_map import shard_map
    from concourse import bass2jax
    from concourse import mybir

    bass2jax.install_neuronx_cc_hook()
    nc = _get_nc(rep)

    partition_name = (nc.partition_id_tensor.name
                      if nc.partition_id_tensor else None)
    in_names, out_names, out_avals = [], [], []
    zero_shapes = []
    for alloc in nc.m.functions[0].allocations:
        if not isinstance(alloc, mybir.MemoryLocationSet):
            continue
        name = alloc.memorylocations[0].name
        if alloc.kind == "ExternalInput":
            if name != partition_name:
                in_names.append(name)
        elif alloc.kind == "ExternalOutput":
            out_names.append(name)
            shape = tuple(alloc.tensor_shape)
            dtype = mybir.dt.np(alloc.dtype)
            out_avals.append(jax.core.ShapedArray(shape, dtype))
            zero_shapes.append((shape, dtype))
    n_params = len(in_names)
    all_names = list(in_names) + out_names
    if partition_name is not None:
        all_names.append(partition_name)

    def _body(*args):
        operands = list(args)
        if partition_name is not None:
            operands.append(bass2jax.partition_id_tensor())
        outs = bass2jax._bass_exec_p.bind(
            *operands,
            out_avals=tuple(out_avals),
            in_names=tuple(all_names),
            out_names=tuple(out_names),
            lowering_input_output_aliases=(),
            sim_require_finite=True,
            sim_require_nnan=True,
            nc=nc,
        )
        return tuple(outs)

    n_cores = len(in_maps)
    devices = jax.devices()[:n_cores]
    mesh = Mesh(np_.asarray(devices), ("core",))
    nzero = len(zero_shapes)
    in_specs = (PartitionSpec("core"),) * (n_params + nzero)
    out_specs = (PartitionSpec("core"),) * len(out_names)
    donate = tuple(range(n_params, n_params + nzero))
    sharded = jax.jit(
        shard_map(_body, mesh=mesh, in_specs=in_specs,
                  out_specs=out_specs, check_rep=False),
        donate_argnums=donate, keep_unused=True)

    concat_in = [
        jax.device_put(
            np_.concatenate([np_.asarray(in_maps[c][name])
                             for c in range(n_cores)], axis=0))
        for name in in_names
    ]

    def make_zeros():
        return [
            jnp.zeros((shape[0] * n_cores,) + tuple(shape[1:]), dtype)
            for shape, dtype in zero_shapes
        ]

    times = []
    for i in range(iters):
        zs = [jax.device_put(z) for z in make_zeros()]
        for z in zs:
            z.block_until_ready()
        t0 = time.perf_counter()
        outs = sharded(*concat_in, *zs)
        for o in outs:
            o.block_until_ready()
        times.append(time.perf_counter() - t0)
    return times


def kernel(x, ln_gamma, ln_beta, W, b):
    x = np.asarray(x, np.float32)
    ln_gamma = np.asarray(ln_gamma, np.float32)
    ln_beta = np.asarray(ln_beta, np.float32)
    W = np.asarray(W, np.float32)
    b = np.asarray(b, np.float32)

    in_maps = _make_in_maps(x, ln_gamma, ln_beta, W, b)
    res = _run(in_maps, trace=False)
    outs = [np.asarray(r["out"]).astype(np.float32) for r in res.results]
    full = np.concatenate(outs, axis=0)  # [1024, 32768]
    return full.reshape(1024, 8, 64, 64)


# revision 11
# speedup vs baseline: 1.2800x; 1.2800x over previous
"""Trainium2 Bass kernel for nn_GroupedLinear (16-group LayerNorm+Linear).

Problem: x [1024, 8, 64, 64] fp32; per group g (16 groups of 64 channels):
  X_g = contiguous 2M-element chunk g viewed row-major as [32768, 64]
  Y_g = LayerNorm(X_g) * gamma_g + beta_g  @ W_g^T + b_g      [32768, 64]
  out chunk g = Y_g^T  (contiguous [64, 32768] block of the output)

Sharding: expert-parallel, 2 groups per core across 8 cores; no collectives.

v4: bf16 wire format both ways; host pre-interleaves the two groups
channel-wise (x_prep[row, (c,g)]) so every device-side access pattern is
contiguous: bn_stats reads [p,128] unit-stride (even/odd stream = g0/g1),
the normalize runs in DVE 2x mode, transposes read contiguous stripes, and
the weight matrix is row-permuted to match the (c,g) contraction order.
Output is written in PSUM column order (contiguous bf16) and the host
inverts the known column permutation during unshard. Engine split per
2048-row macro: DVE = 16x bn_stats + rstd-multiply (2x mode) + one
psum->sbuf job; Pool = mean subtract; ACT = rstd/mu prep + remaining
psum->sbuf copies; PE = 16 transposes + 2 N=1024 matmuls.
"""

import sys

for _p in ("/opt/trn_rl_repo", "/opt/pypackages"):
    if _p not in sys.path:
        sys.path.insert(0, _p)

import numpy as np
import ml_dtypes

G_TOTAL = 16
N_CORES = 8
G_PER_CORE = G_TOTAL // N_CORES  # 2
IN_G = 64
OUT_G = 64
K = G_PER_CORE * IN_G  # 128 interleaved (c,g) channels
ROWS = 8 * 64 * 64  # 32768 rows per group
MACRO = 2048  # rows per macro-tile
NB = MACRO // 128  # 16 row-blocks per macro (row = p*NB + b)
NMAC = ROWS // MACRO  # 16
EPS = 1e-6

_CACHE = {}
_LAST_RESULTS = None


def _build_bass(rep=1):
    import concourse.bacc as bacc
    import concourse.bass as bass
    import concourse.tile as tile
    from concourse import mybir

    nc = bacc.Bacc(None, target_bir_lowering=False)

    x = nc.dram_tensor("x", [ROWS, K], mybir.dt.bfloat16,
                       kind="ExternalInput")
    wb = nc.dram_tensor("wb", [128, 128], mybir.dt.bfloat16,
                        kind="ExternalInput")
    tb = nc.dram_tensor("tb", [128, 1], mybir.dt.float32,
                        kind="ExternalInput")
    ident = nc.dram_tensor("ident", [128, 128], mybir.dt.bfloat16,
                           kind="ExternalInput")
    out = nc.dram_tensor("out", [128, ROWS], mybir.dt.bfloat16,
                         kind="ExternalOutput")

    F = mybir.ActivationFunctionType

    with tile.TileContext(nc) as tc:
        with (
            tc.tile_pool(name="singles", bufs=1) as singles,
            tc.tile_pool(name="xload", bufs=3) as xload,
            tc.tile_pool(name="statp", bufs=3) as statp,
            tc.tile_pool(name="rstdp", bufs=3) as rstdp,
            tc.tile_pool(name="mup", bufs=3) as mup,
            tc.tile_pool(name="xnp", bufs=2) as xnp,
            tc.tile_pool(name="xtsp", bufs=3) as xtsp,
            tc.tile_pool(name="youtp", bufs=2) as youtp,
            tc.tile_pool(name="xtpp", bufs=2, space="PSUM") as xtpp,
            tc.tile_pool(name="ypp", bufs=2, space="PSUM") as ypp,
        ):
            sb_wb = singles.tile([128, 128], mybir.dt.bfloat16)
            sb_tb = singles.tile([128, 1], mybir.dt.float32)
            sb_id = singles.tile([128, 128], mybir.dt.bfloat16)
            sb_eps = singles.tile([128, 1], mybir.dt.float32)
            nc.sync.dma_start(out=sb_wb, in_=wb[:, :])
            nc.sync.dma_start(out=sb_tb, in_=tb[:, :])
            nc.sync.dma_start(out=sb_id, in_=ident[:, :])
            nc.vector.memset(sb_eps, EPS)

            for m in range(NMAC * rep):
                m = m % NMAC
                r0 = m * MACRO
                # ---- load: partition p holds rows NB*p .. NB*p+NB-1, all
                # 128 interleaved channels -> one 4KB contiguous run per
                # partition
                x_t = xload.tile([128, NB, K], mybir.dt.bfloat16)
                nc.sync.dma_start(
                    out=x_t,
                    in_=x[r0:r0 + MACRO, :].rearrange(
                        "(p b) k -> p b k", p=128),
                )

                # ---- stats: 16 bn_stats on contiguous [p, 128] streams;
                # (c,g) interleave makes even positions g0, odd g1 ->
                # out [p, 6] = [cnt0, mu0, M2_0, cnt1, mu1, M2_1]
                st = statp.tile([128, NB, 6], mybir.dt.float32)
                for bb in range(NB):
                    nc.vector.add_instruction(
                        mybir.InstBNStats(
                            name=nc.get_next_instruction_name(),
                            ins=[nc.vector.lower_ap(x_t[:, bb, :])],
                            outs=[nc.vector.lower_ap(st[:, bb, :])],
                        )
                    )
                # rstd2[p, b, g] = 1/sqrt(M2/64+eps) bf16 (g contiguous)
                rstd2 = rstdp.tile([128, NB, G_PER_CORE], mybir.dt.bfloat16)
                st_ap = st[:, :, :]
                m2_in = bass.AP(
                    tensor=st_ap.tensor, offset=st_ap.offset + 2,
                    ap=[st_ap.ap[0], [6, NB], [3, G_PER_CORE]],
                )
                nc.scalar.activation(out=rstd2, in_=m2_in,
                                     func=F.Abs_reciprocal_sqrt,
                                     bias=sb_eps[:, 0:1],
                                     scale=1.0 / IN_G)

                # ---- normalize: sub on GpSimd (fp32 mu broadcast), mul on
                # DVE in 2x mode (all-bf16, unit-stride innermost g-pairs)
                xn = xnp.tile([128, NB, K], mybir.dt.bfloat16)
                xn_bcg = bass.AP(
                    tensor=xn.tensor, offset=xn[:, :, :].offset,
                    ap=[xn[:, :, :].ap[0], [K, NB], [2, IN_G], [1, 2]],
                )
                xt_bcg = bass.AP(
                    tensor=x_t.tensor, offset=x_t[:, :, :].offset,
                    ap=[x_t[:, :, :].ap[0], [K, NB], [2, IN_G], [1, 2]],
                )
                mu_b = bass.AP(
                    tensor=st_ap.tensor, offset=st_ap.offset + 1,
                    ap=[st_ap.ap[0], [6, NB], [0, IN_G], [3, G_PER_CORE]],
                )
                nc.gpsimd.tensor_sub(xn_bcg, xt_bcg, mu_b)
                rstd_b = bass.AP(
                    tensor=rstd2.tensor, offset=rstd2[:, :, :].offset,
                    ap=[rstd2[:, :, :].ap[0], [2, NB], [0, IN_G], [1, 2]],
                )
                nc.vector.tensor_mul(xn_bcg, xn_bcg, rstd_b)

                # ---- per half-macro: 8 transposes -> PSUM, copy -> SBUF,
                # one N=1024 matmul -> PSUM f32, bias-add -> contiguous
                # bf16 (PSUM column order; host inverts the permutation)
                y_t = youtp.tile([128, MACRO], mybir.dt.bfloat16)
                for h in range(2):
                    xtp = xtpp.tile([128, 1024], mybir.dt.bfloat16)
                    for s in range(8):
                        nc.tensor.transpose(
                            out=xtp[:, s * 128:(s + 1) * 128],
                            in_=xn[:, 8 * h + s, :],
                            identity=sb_id,
                        )
                    xts = xtsp.tile([128, 1024], mybir.dt.bfloat16)
                    if h == 0:
                        nc.scalar.activation(out=xts, in_=xtp, func=F.Copy)
                    else:
                        nc.vector.tensor_copy(xts, xtp)
                    yp = ypp.tile([128, 1024], mybir.dt.float32)
                    for j in range(2):
                        nc.tensor.matmul(yp[:, j * 512:(j + 1) * 512],
                                         lhsT=sb_wb,
                                         rhs=xts[:, j * 512:(j + 1) * 512],
                                         start=True, stop=True)
                    yt_v = y_t[:, h * 1024:(h + 1) * 1024]
                    if h == 0:
                        nc.vector.tensor_scalar_add(yt_v, yp, sb_tb[:, 0:1])
                    else:
                        nc.scalar.activation(out=yt_v, in_=yp,
                                             func=F.Identity,
                                             bias=sb_tb[:, 0:1], scale=1.0)

                nc.sync.dma_start(out=out[:, r0:r0 + MACRO], in_=y_t)

    nc.finalize()
    return nc


def _get_nc(rep=1):
    key = ("nc", rep)
    if key not in _CACHE:
        _CACHE[key] = _build_bass(rep)
    return _CACHE[key]


def _make_in_maps(x, ln_gamma, ln_beta, W, b):
    bf16 = ml_dtypes.bfloat16
    xg = x.reshape(G_TOTAL, ROWS, IN_G)
    ident_bf = np.eye(128, dtype=np.float32).astype(bf16)
    # channel permutation: device k = c*2 + g  <- source (g, c)
    perm = np.empty(128, np.int64)
    for g in range(G_PER_CORE):
        for c in range(IN_G):
            perm[c * G_PER_CORE + g] = g * IN_G + c
    in_maps = []
    for core in range(N_CORES):
        gs = [G_PER_CORE * core + g for g in range(G_PER_CORE)]
        wbc = np.zeros((128, 128), np.float32)
        tbc = np.zeros((128, 1), np.float32)
        for g_local, g in enumerate(gs):
            Wp = W[g] * ln_gamma[g][None, :]  # [out, in] gamma folded
            lo = g_local * 64
            wbc[lo:lo + 64, lo:lo + 64] = Wp.T  # lhsT[k=in, m=out]
            tbc[lo:lo + 64, 0] = W[g] @ ln_beta[g] + b[g]
        wbc = wbc[perm, :]  # rows now in interleaved (c,g) order
        # x interleaved: [ROWS, (c,g)]
        xi = np.ascontiguousarray(
            xg[gs[0]:gs[-1] + 1].transpose(1, 2, 0).reshape(ROWS, K)
        ).astype(bf16)
        in_maps.append({
            "x": xi,
            "wb": wbc.astype(bf16),
            "tb": tbc,
            "ident": ident_bf,
        })
    return in_maps


def _unpermute(dev_out):
    """Invert the PSUM column order: flat = m*2048 + h*1024 + s*128 + q
    maps to row m*2048 + q*16 + 8h + s."""
    a = dev_out.reshape(128, NMAC, 2, 8, 128)  # [p, m, h, s, q]
    a = a.transpose(0, 1, 4, 2, 3)  # [p, m, q, h, s]
    return np.ascontiguousarray(a).reshape(128, ROWS)


def _run(in_maps, trace=False):
    from concourse.bass_utils import run_bass_kernel_spmd
    global _LAST_RESULTS
    nc = _get_nc()
    res = run_bass_kernel_spmd(nc, in_maps, list(range(N_CORES)),
                               trace=trace)
    _LAST_RESULTS = res
    return res


def kernel(x, ln_gamma, ln_beta, W, b):
    x = np.asarray(x, np.float32)
    ln_gamma = np.asarray(ln_gamma, np.float32)
    ln_beta = np.asarray(ln_beta, np.float32)
    W = np.asarray(W, np.float32)
    b = np.asarray(b, np.float32)

    in_maps = _make_in_maps(x, ln_gamma, ln_beta, W, b)
    res = _run(in_maps, trace=False)
    outs = [_unpermute(np.asarray(r["out"])).astype(np.float32)
            for r in res.results]
    full = np.concatenate(outs, axis=0)  # [1024, 32768]
    return full.reshape(1024, 8, 64, 64)


# revision 13
# speedup vs baseline: 1.7094x; 1.3355x over previous
"""Trainium2 Bass kernel for nn_GroupedLinear (16-group LayerNorm+Linear).

Problem: x [1024, 8, 64, 64] fp32; per group g (16 groups of 64 channels):
  X_g = contiguous 2M-element chunk g viewed row-major as [32768, 64]
  Y_g = LayerNorm(X_g) * gamma_g + beta_g  @ W_g^T + b_g      [32768, 64]
  out chunk g = Y_g^T  (contiguous [64, 32768] block of the output)

Sharding: expert-parallel, 2 groups per core across 8 cores; no collectives.

v4: bf16 wire format both ways; host pre-interleaves the two groups
channel-wise (x_prep[row, (c,g)]) so every device-side access pattern is
contiguous: bn_stats reads [p,128] unit-stride (even/odd stream = g0/g1),
the normalize runs in DVE 2x mode, transposes read contiguous stripes, and
the weight matrix is row-permuted to match the (c,g) contraction order.
Output is written in PSUM column order (contiguous bf16) and the host
inverts the known column permutation during unshard. Engine split per
2048-row macro: DVE = 16x bn_stats + rstd-multiply (2x mode) + one
psum->sbuf job; Pool = mean subtract; ACT = rstd/mu prep + remaining
psum->sbuf copies; PE = 16 transposes + 2 N=1024 matmuls.
"""

import sys

for _p in ("/opt/trn_rl_repo", "/opt/pypackages"):
    if _p not in sys.path:
        sys.path.insert(0, _p)

import numpy as np
import ml_dtypes

G_TOTAL = 16
N_CORES = 8
G_PER_CORE = G_TOTAL // N_CORES  # 2
IN_G = 64
OUT_G = 64
K = G_PER_CORE * IN_G  # 128 interleaved (c,g) channels
ROWS = 8 * 64 * 64  # 32768 rows per group
MACRO = 2048  # rows per macro-tile
NB = MACRO // 128  # 16 row-blocks per macro (row = p*NB + b)
NMAC = ROWS // MACRO  # 16
EPS = 1e-6

_CACHE = {}
_LAST_RESULTS = None


def _build_bass(rep=1):
    import concourse.bacc as bacc
    import concourse.bass as bass
    import concourse.tile as tile
    from concourse import mybir

    nc = bacc.Bacc(None, target_bir_lowering=False)

    x = nc.dram_tensor("x", [ROWS, K], mybir.dt.bfloat16,
                       kind="ExternalInput")
    wb = nc.dram_tensor("wb", [128, 128], mybir.dt.bfloat16,
                        kind="ExternalInput")
    tb = nc.dram_tensor("tb", [128, 1], mybir.dt.float32,
                        kind="ExternalInput")
    ident = nc.dram_tensor("ident", [128, 128], mybir.dt.bfloat16,
                           kind="ExternalInput")
    out = nc.dram_tensor("out", [128, ROWS], mybir.dt.bfloat16,
                         kind="ExternalOutput")

    F = mybir.ActivationFunctionType

    with tile.TileContext(nc) as tc:
        with (
            tc.tile_pool(name="singles", bufs=1) as singles,
            tc.tile_pool(name="xload", bufs=4) as xload,
            tc.tile_pool(name="statp", bufs=4) as statp,
            tc.tile_pool(name="rstdp", bufs=4) as rstdp,
            tc.tile_pool(name="xnp", bufs=3) as xnp,
            tc.tile_pool(name="xtsp", bufs=4) as xtsp,
            tc.tile_pool(name="youtp", bufs=3) as youtp,
            tc.tile_pool(name="xtpp", bufs=2, space="PSUM") as xtpp,
            tc.tile_pool(name="ypp", bufs=3, space="PSUM") as ypp,
        ):
            sb_wb = singles.tile([128, 128], mybir.dt.bfloat16)
            sb_tb = singles.tile([128, 1], mybir.dt.float32)
            sb_id = singles.tile([128, 128], mybir.dt.bfloat16)
            sb_eps = singles.tile([128, 1], mybir.dt.float32)
            nc.sync.dma_start(out=sb_wb, in_=wb[:, :])
            nc.sync.dma_start(out=sb_tb, in_=tb[:, :])
            nc.sync.dma_start(out=sb_id, in_=ident[:, :])
            nc.vector.memset(sb_eps, EPS)

            for m in range(NMAC * rep):
                m = m % NMAC
                r0 = m * MACRO
                # ---- load: partition p holds rows NB*p .. NB*p+NB-1, all
                # 128 interleaved channels -> one 4KB contiguous run per
                # partition
                x_t = xload.tile([128, NB, K], mybir.dt.bfloat16)
                nc.sync.dma_start(
                    out=x_t,
                    in_=x[r0:r0 + MACRO, :].rearrange(
                        "(p b) k -> p b k", p=128),
                )

                # ---- stats: 16 bn_stats on contiguous [p, 128] streams;
                # (c,g) interleave makes even positions g0, odd g1 ->
                # out [p, 6] = [cnt0, mu0, M2_0, cnt1, mu1, M2_1]
                st = statp.tile([128, NB, 6], mybir.dt.float32)
                for bb in range(NB):
                    nc.vector.add_instruction(
                        mybir.InstBNStats(
                            name=nc.get_next_instruction_name(),
                            ins=[nc.vector.lower_ap(x_t[:, bb, :])],
                            outs=[nc.vector.lower_ap(st[:, bb, :])],
                        )
                    )
                # rstd2[p, b, g] = 1/sqrt(M2/64+eps) bf16 (g contiguous)
                rstd2 = rstdp.tile([128, NB, G_PER_CORE], mybir.dt.bfloat16)
                st_ap = st[:, :, :]
                m2_in = bass.AP(
                    tensor=st_ap.tensor, offset=st_ap.offset + 2,
                    ap=[st_ap.ap[0], [6, NB], [3, G_PER_CORE]],
                )
                nc.scalar.activation(out=rstd2, in_=m2_in,
                                     func=F.Abs_reciprocal_sqrt,
                                     bias=sb_eps[:, 0:1],
                                     scale=1.0 / IN_G)

                # ---- normalize: sub on GpSimd (fp32 mu broadcast), mul on
                # DVE in 2x mode (all-bf16, unit-stride innermost g-pairs)
                xn = xnp.tile([128, NB, K], mybir.dt.bfloat16)
                xn_bcg = bass.AP(
                    tensor=xn.tensor, offset=xn[:, :, :].offset,
                    ap=[xn[:, :, :].ap[0], [K, NB], [2, IN_G], [1, 2]],
                )
                xt_bcg = bass.AP(
                    tensor=x_t.tensor, offset=x_t[:, :, :].offset,
                    ap=[x_t[:, :, :].ap[0], [K, NB], [2, IN_G], [1, 2]],
                )
                mu_b = bass.AP(
                    tensor=st_ap.tensor, offset=st_ap.offset + 1,
                    ap=[st_ap.ap[0], [6, NB], [0, IN_G], [3, G_PER_CORE]],
                )
                nc.gpsimd.tensor_sub(xn_bcg, xt_bcg, mu_b)
                rstd_b = bass.AP(
                    tensor=rstd2.tensor, offset=rstd2[:, :, :].offset,
                    ap=[rstd2[:, :, :].ap[0], [2, NB], [0, IN_G], [1, 2]],
                )
                nc.vector.tensor_mul(xn_bcg, xn_bcg, rstd_b)

                # ---- per half-macro: 8 transposes -> PSUM, copy -> SBUF,
                # one N=1024 matmul -> PSUM f32, bias-add -> contiguous
                # bf16 (PSUM column order; host inverts the permutation)
                y_t = youtp.tile([128, MACRO], mybir.dt.bfloat16)
                for h in range(2):
                    xtp = xtpp.tile([128, 1024], mybir.dt.bfloat16)
                    for s in range(8):
                        nc.tensor.transpose(
                            out=xtp[:, s * 128:(s + 1) * 128],
                            in_=xn[:, 8 * h + s, :],
                            identity=sb_id,
                        )
                    xts = xtsp.tile([128, 1024], mybir.dt.bfloat16)
                    nc.scalar.activation(out=xts, in_=xtp, func=F.Copy)
                    yp = ypp.tile([128, 1024], mybir.dt.float32)
                    for j in range(2):
                        nc.tensor.matmul(yp[:, j * 512:(j + 1) * 512],
                                         lhsT=sb_wb,
                                         rhs=xts[:, j * 512:(j + 1) * 512],
                                         start=True, stop=True)
                    yt_v = y_t[:, h * 1024:(h + 1) * 1024]
                    nc.scalar.activation(out=yt_v, in_=yp,
                                         func=F.Identity,
                                         bias=sb_tb[:, 0:1], scale=1.0)

                # out-DMA on the scalar HWDGE ring so the sync ring's FIFO
                # (carrying the input loads) never blocks behind it
                nc.scalar.dma_start(out=out[:, r0:r0 + MACRO], in_=y_t)

    nc.finalize()
    return nc


def _get_nc(rep=1):
    key = ("nc", rep)
    if key not in _CACHE:
        _CACHE[key] = _build_bass(rep)
    return _CACHE[key]


def _make_in_maps(x, ln_gamma, ln_beta, W, b):
    bf16 = ml_dtypes.bfloat16
    xg = x.reshape(G_TOTAL, ROWS, IN_G)
    ident_bf = np.eye(128, dtype=np.float32).astype(bf16)
    # channel permutation: device k = c*2 + g  <- source (g, c)
    perm = np.empty(128, np.int64)
    for g in range(G_PER_CORE):
        for c in range(IN_G):
            perm[c * G_PER_CORE + g] = g * IN_G + c
    in_maps = []
    for core in range(N_CORES):
        gs = [G_PER_CORE * core + g for g in range(G_PER_CORE)]
        wbc = np.zeros((128, 128), np.float32)
        tbc = np.zeros((128, 1), np.float32)
        for g_local, g in enumerate(gs):
            Wp = W[g] * ln_gamma[g][None, :]  # [out, in] gamma folded
            lo = g_local * 64
            wbc[lo:lo + 64, lo:lo + 64] = Wp.T  # lhsT[k=in, m=out]
            tbc[lo:lo + 64, 0] = W[g] @ ln_beta[g] + b[g]
        wbc = wbc[perm, :]  # rows now in interleaved (c,g) order
        # x interleaved: [ROWS, (c,g)]
        xi = np.ascontiguousarray(
            xg[gs[0]:gs[-1] + 1].transpose(1, 2, 0).reshape(ROWS, K)
        ).astype(bf16)
        in_maps.append({
            "x": xi,
            "wb": wbc.astype(bf16),
            "tb": tbc,
            "ident": ident_bf,
        })
    return in_maps


def _unpermute(dev_out):
    """Invert the PSUM column order: flat = m*2048 + h*1024 + s*128 + q
    maps to row m*2048 + q*16 + 8h + s."""
    a = dev_out.reshape(128, NMAC, 2, 8, 128)  # [p, m, h, s, q]
    a = a.transpose(0, 1, 4, 2, 3)  # [p, m, q, h, s]
    return np.ascontiguousarray(a).reshape(128, ROWS)


def _run(in_maps, trace=False):
    from concourse.bass_utils import run_bass_kernel_spmd
    global _LAST_RESULTS
    nc = _get_nc()
    res = run_bass_kernel_spmd(nc, in_maps, list(range(N_CORES)),
                               trace=trace)
    _LAST_RESULTS = res
    return res


def kernel(x, ln_gamma, ln_beta, W, b):
    x = np.asarray(x, np.float32)
    ln_gamma = np.asarray(ln_gamma, np.float32)
    ln_beta = np.asarray(ln_beta, np.float32)
    W = np.asarray(W, np.float32)
    b = np.asarray(b, np.float32)

    in_maps = _make_in_maps(x, ln_gamma, ln_beta, W, b)
    res = _run(in_maps, trace=False)
    outs = [_unpermute(np.asarray(r["out"])).astype(np.float32)
            for r in res.results]
    full = np.concatenate(outs, axis=0)  # [1024, 32768]
    return full.reshape(1024, 8, 64, 64)
